# revision 8
# baseline (speedup 1.0000x reference)
"""KMeans (B=4, N=8192, D=256, K=512, 10 Lloyd iterations) on 8 trn2 cores.

Sharding: core c handles batch b = c//2 and cluster half h = c%2 (256
clusters each).  Each core scores all 8192 points of its batch against its
256 clusters, takes a local argmax of (x.c - |c|^2/2), and the two cores of
a batch exchange (maxval, label) via a pairwise AllGather to form the global
argmin labels.  Each core then accumulates per-cluster sums/counts for its
own clusters with a one-hot matmul (counts fused as an extra ones column)
and updates its centroids locally — no all-reduce needed.

Outputs: labels (4, 8192) int32 and centers (4, 512, 256) float32, matching
jnp.argmin tie-breaking (lowest index; cross-half ties resolve to the low
half).
"""

import sys

sys.path.insert(0, "/opt/trn_rl_repo")

import numpy as np

import concourse.bass as bass
import concourse.mybir as mybir
import concourse.tile as tile
from concourse.bass_utils import run_bass_kernel_spmd

F32 = mybir.dt.float32
I32 = mybir.dt.int32
U32 = mybir.dt.uint32

B, N, D, K = 4, 8192, 256, 512
KH = K // 2          # clusters per core
P = 128
NT = N // P          # 64 n-tiles of 128 points
DC = D // P          # 2 contraction chunks
ITERS = 10
N_CORES = 8
REPLICA_GROUPS = [[0, 1], [2, 3], [4, 5], [6, 7]]


def _split_multiwait_insts(nc):
    """This walrus build rejects instructions with >1 semaphore wait
    ("Too many sync wait commands").  Hoist extra waits onto single-wait
    NoOps on the same engine immediately before the instruction —
    semantically identical."""
    n_new = 0
    for f in nc.m.functions:
        for bb in f.blocks:
            out = []
            changed = False
            for inst in bb.instructions:
                si = getattr(inst, "sync_info", None)
                if si is not None and len(si.on_wait) > 1:
                    waits = list(si.on_wait)
                    ups = list(si.on_update)
                    for k, w in enumerate(waits[:-1]):
                        nop = mybir.InstNoOp(
                            name=f"{inst.name}-hw{k}", ins=[], outs=[]
                        )
                        nop.engine = inst.engine
                        nop.sync_info = mybir.SyncInfo(on_wait=[w], on_update=[])
                        out.append(nop)
                        n_new += 1
                    inst.sync_info = mybir.SyncInfo(
                        on_wait=[waits[-1]], on_update=ups
                    )
                    changed = True
                out.append(inst)
            if changed:
                bb.instructions = out
    return n_new


def build():
    nc = bass.Bass("TRN2", num_devices=N_CORES)
    Alu = mybir.AluOpType

    xa_d = nc.dram_tensor("xa", [N, D + 1], F32, kind="ExternalInput")
    xt_d = nc.dram_tensor("xt", [D, N], F32, kind="ExternalInput")
    c0_d = nc.dram_tensor("c0", [KH, D], F32, kind="ExternalInput")
    c0t_d = nc.dram_tensor("c0t", [D, KH], F32, kind="ExternalInput")
    iota_d = nc.dram_tensor("iota", [P, KH], F32, kind="ExternalInput")
    ident_d = nc.dram_tensor("ident", [P, P], F32, kind="ExternalInput")
    labels_d = nc.dram_tensor("labels", [N], I32, kind="ExternalOutput")
    centers_d = nc.dram_tensor("centers", [KH, D], F32, kind="ExternalOutput")

    with tile.TileContext(nc) as tc:
        with (
            tc.tile_pool(name="res", bufs=1) as res,          # persistent state
            tc.tile_pool(name="work", bufs=3) as work,        # score / onehot
            tc.tile_pool(name="small", bufs=2) as small,      # per-iter misc
            tc.tile_pool(name="ps_xc", bufs=3, space="PSUM") as ps_xc,
            tc.tile_pool(name="ps_sum", bufs=1, space="PSUM") as ps_sum,
            tc.tile_pool(name="ps_aux", bufs=1, space="PSUM") as ps_aux,
            tc.tile_pool(name="dram", bufs=2, space="DRAM") as dram,
        ):
            # ---- resident loads ----
            xa_sb = res.tile([P, NT, D + 1], F32, tag="xa")
            # point n = t*P + p lives at xa_sb[p, t, :] — matches the score
            # matmul, whose output partition is the lhsT free index.
            nc.sync.dma_start(xa_sb[:], xa_d.rearrange("(t p) d -> p t d", p=P))
            xt_sb = res.tile([P, DC, N], F32, tag="xt")
            nc.sync.dma_start(xt_sb[:], xt_d.rearrange("(c p) n -> p c n", p=P))
            cT = res.tile([P, DC, KH], F32, tag="cT")
            nc.sync.dma_start(cT[:], c0t_d.rearrange("(c p) k -> p c k", p=P))
            centers = res.tile([P, 2, D], F32, tag="centers")
            nc.sync.dma_start(
                centers[:], c0_d.rearrange("(kt p) d -> p kt d", p=P)
            )
            iota = res.tile([P, KH], F32, tag="iota")
            nc.sync.dma_start(iota[:], iota_d[:])
            ident = res.tile([P, P], F32, tag="ident")
            nc.sync.dma_start(ident[:], ident_d[:])
            half = res.tile([P, P], F32, tag="half")
            nc.gpsimd.memset(half[:], 0.5)
            c2b = res.tile([P, KH], F32, tag="c2b")
            sq = res.tile([P, DC, KH], F32, tag="sq")
            glabel = res.tile([P, NT], F32, tag="glabel")

            def refresh_c2b():
                # c2b[j, k] = 0.5 * sum_d cT[d, k]^2, broadcast over all
                # partitions j via an all-0.5 stationary matmul.
                nc.vector.tensor_tensor(sq[:], cT[:], cT[:], Alu.mult)
                psc = ps_aux.tile([P, KH], F32, tag="psc")
                nc.tensor.matmul(
                    psc[:], half[:], sq[:, 0, :], start=True, stop=False
                )
                nc.tensor.matmul(
                    psc[:], half[:], sq[:, 1, :], start=False, stop=True
                )
                nc.vector.tensor_copy(c2b[:], psc[:])

            refresh_c2b()

            for it in range(ITERS + 1):
                # ---- phase A: scores + local argmax ----
                mx8 = small.tile([P, NT, 8], F32, tag="mx8")
                ix8 = small.tile([P, NT, 8], U32, tag="ix8")
                for t in range(NT):
                    psa = ps_xc.tile([P, KH], F32, tag="psa")
                    nc.tensor.matmul(
                        psa[:],
                        xt_sb[:, 0, t * P:(t + 1) * P],
                        cT[:, 0, :],
                        start=True,
                        stop=False,
                    )
                    nc.tensor.matmul(
                        psa[:],
                        xt_sb[:, 1, t * P:(t + 1) * P],
                        cT[:, 1, :],
                        start=False,
                        stop=True,
                    )
                    score = work.tile([P, KH], F32, tag="score")
                    nc.vector.tensor_tensor(
                        score[:], psa[:], c2b[:], Alu.subtract
                    )
                    nc.vector.max(mx8[:, t, :], score[:])
                    nc.vector.max_index(ix8[:, t, :], mx8[:, t, :], score[:])

                # ---- phase B: pairwise exchange of (maxval, global label) ----
                xch = small.tile([P, 2, NT], F32, tag="xch")
                nc.vector.tensor_copy(xch[:, 0, :], mx8[:, :, 0])
                lidx = small.tile([P, NT], F32, tag="lidx")
                nc.vector.tensor_copy(lidx[:], ix8[:, :, 0])
                # local -> global cluster index (iota[:, 0] is koff)
                nc.vector.tensor_scalar(
                    xch[:, 1, :], lidx[:], iota[:, 0:1], None, Alu.add
                )
                in_b = dram.tile([2, N], F32, tag="in_b")
                out_b = dram.tile([4, N], F32, tag="out_b")
                nc.sync.dma_start(
                    in_b[:].rearrange("r (t p) -> p r t", p=P), xch[:]
                )
                nc.gpsimd.collective_compute(
                    "AllGather",
                    Alu.bypass,
                    replica_groups=REPLICA_GROUPS,
                    ins=[in_b.opt()],
                    outs=[out_b.opt()],
                )
                pair = small.tile([P, 4, NT], F32, tag="pair")
                nc.sync.dma_start(
                    pair[:], out_b[:].rearrange("r (t p) -> p r t", p=P)
                )
                # global argmax: half 1 wins only on strictly greater score,
                # so exact ties resolve to the lower cluster index.
                mask = small.tile([P, NT], I32, tag="mask")
                nc.vector.tensor_tensor(
                    mask[:], pair[:, 2, :], pair[:, 0, :], Alu.is_gt
                )
                nc.vector.select(
                    glabel[:], mask[:], pair[:, 3, :], pair[:, 1, :]
                )

                if it == ITERS:
                    li = small.tile([P, NT], I32, tag="li")
                    nc.vector.tensor_copy(li[:], glabel[:])
                    nc.sync.dma_start(
                        labels_d.rearrange("(t p) -> p t", p=P), li[:]
                    )
                    nc.sync.dma_start(
                        centers_d.rearrange("(kt p) d -> p kt d", p=P),
                        centers[:],
                    )
                    break

                # ---- phase C: one-hot sums (counts fused as ones column) ----
                pss = [
                    ps_sum.tile([P, D + 1], F32, tag=f"pss{kt}", name=f"pss{kt}")
                    for kt in range(2)
                ]
                for t in range(NT):
                    oh = work.tile([P, KH], F32, tag="oh")
                    nc.vector.tensor_scalar(
                        oh[:], iota[:], glabel[:, t:t + 1], None, Alu.is_equal
                    )
                    for kt in range(2):
                        nc.tensor.matmul(
                            pss[kt][:],
                            oh[:, kt * P:(kt + 1) * P],
                            xa_sb[:, t, :],
                            start=(t == 0),
                            stop=(t == NT - 1),
                        )

                # ---- phase D: centroid update ----
                for kt in range(2):
                    counts = pss[kt][:, D:D + 1]
                    cnt1 = small.tile([P, 1], F32, tag="cnt1")
                    nc.vector.tensor_scalar(
                        cnt1[:], counts, 1.0, None, Alu.max
                    )
                    recip = small.tile([P, 1], F32, tag="recip")
                    nc.vector.reciprocal(recip[:], cnt1[:])
                    maskc = small.tile([P, 1], F32, tag="maskc")
                    nc.vector.tensor_scalar(
                        maskc[:], counts, 0.0, None, Alu.is_gt
                    )
                    nmask = small.tile([P, 1], F32, tag="nmask")
                    nc.vector.tensor_scalar(
                        nmask[:], counts, 0.0, None, Alu.is_le
                    )
                    # new = sums/max(counts,1) * (counts>0) + old * (counts<=0)
                    cnew = small.tile([P, D], F32, tag="cnew")
                    nc.vector.tensor_scalar(
                        cnew[:], pss[kt][:, 0:D], recip[:], maskc[:],
                        Alu.mult, Alu.mult,
                    )
                    cold = small.tile([P, D], F32, tag="cold")
                    nc.vector.tensor_scalar(
                        cold[:], centers[:, kt, :], nmask[:], None, Alu.mult
                    )
                    nc.vector.tensor_tensor(
                        centers[:, kt, :], cnew[:], cold[:], Alu.add
                    )

                # rebuild cT (transpose) and c2b for the next iteration
                for kt in range(2):
                    for dc in range(DC):
                        pst = ps_aux.tile([P, P], F32, tag="pst")
                        nc.tensor.transpose(
                            pst[:],
                            centers[:, kt, dc * P:(dc + 1) * P],
                            ident[:],
                        )
                        nc.vector.tensor_copy(
                            cT[:, dc, kt * P:(kt + 1) * P], pst[:]
                        )
                refresh_c2b()

    _split_multiwait_insts(nc)
    return nc


def kernel(x):
    x = np.ascontiguousarray(np.asarray(x, dtype=np.float32))
    assert x.shape == (B, N, D)
    nc = build()

    in_maps = []
    ident = np.eye(P, dtype=np.float32)
    for c in range(N_CORES):
        b, h = c // 2, c % 2
        xb = x[b]
        koff = h * KH
        in_maps.append(
            {
                "xa": np.ascontiguousarray(
                    np.concatenate(
                        [xb, np.ones((N, 1), np.float32)], axis=1
                    )
                ),
                "xt": np.ascontiguousarray(xb.T),
                "c0": np.ascontiguousarray(xb[koff:koff + KH]),
                "c0t": np.ascontiguousarray(xb[koff:koff + KH].T),
                "iota": np.tile(
                    np.arange(koff, koff + KH, dtype=np.float32), (P, 1)
                ),
                "ident": ident,
            }
        )

    res = run_bass_kernel_spmd(nc, in_maps, core_ids=list(range(N_CORES)))
    labels = np.stack(
        [res.results[2 * b]["labels"].reshape(N) for b in range(B)]
    ).astype(np.int32)
    centers = np.stack(
        [
            np.concatenate(
                [
                    res.results[2 * b]["centers"],
                    res.results[2 * b + 1]["centers"],
                ],
                axis=0,
            )
            for b in range(B)
        ]
    ).astype(np.float32)
    return labels, centers


# revision 23
# speedup vs baseline: 43.8852x; 43.8852x over previous
"""KMeans (B=4, N=8192, D=256, K=512, 10 Lloyd iterations) on 8 trn2 cores.

Sharding: core c handles batch b = c//2 and cluster half h = c%2 (256
clusters each).  Each core scores all 8192 points of its batch against its
256 clusters, takes a local argmax of (x.c - |c|^2/2), and the two cores of
a batch exchange (maxval, label) via a pairwise AllGather to form the global
argmin labels.  Each core then accumulates per-cluster sums/counts for its
own clusters with a one-hot matmul (counts fused as an extra ones column)
and updates its centroids locally — no all-reduce needed.

Outputs: labels (4, 8192) int32 and centers (4, 512, 256) float32, matching
jnp.argmin tie-breaking (lowest index; cross-half ties resolve to the low
half).
"""

import sys

sys.path.insert(0, "/opt/trn_rl_repo")

import numpy as np

import concourse.bass as bass
import concourse.mybir as mybir
import concourse.tile as tile
from concourse.bass_utils import run_bass_kernel_spmd

F32 = mybir.dt.float32
I32 = mybir.dt.int32
U32 = mybir.dt.uint32

B, N, D, K = 4, 8192, 256, 512
KH = K // 2          # clusters per core
P = 128
NT = N // P          # 64 n-tiles of 128 points
DC = D // P          # 2 contraction chunks
ITERS = 10
N_CORES = 8
REPLICA_GROUPS = [[0, 1], [2, 3], [4, 5], [6, 7]]


def _split_multiwait_insts(nc):
    """This walrus build rejects instructions with >1 semaphore wait
    ("Too many sync wait commands").  Hoist extra waits onto single-wait
    NoOps on the same engine immediately before the instruction —
    semantically identical."""
    n_new = 0
    for f in nc.m.functions:
        for bb in f.blocks:
            out = []
            changed = False
            for inst in bb.instructions:
                si = getattr(inst, "sync_info", None)
                if si is not None and len(si.on_wait) > 1:
                    waits = list(si.on_wait)
                    ups = list(si.on_update)
                    for k, w in enumerate(waits[:-1]):
                        nop = mybir.InstNoOp(
                            name=f"{inst.name}-hw{k}", ins=[], outs=[]
                        )
                        nop.engine = inst.engine
                        nop.sync_info = mybir.SyncInfo(on_wait=[w], on_update=[])
                        out.append(nop)
                        n_new += 1
                    inst.sync_info = mybir.SyncInfo(
                        on_wait=[waits[-1]], on_update=ups
                    )
                    changed = True
                out.append(inst)
            if changed:
                bb.instructions = out
    return n_new


def build(iters=ITERS, use_collective=True, mm_r=False, mm_r_sums=None,
          dbg_skip_exchange=False, dbg_skip_out=False, dbg_skip_cd=False):
    nc = bass.Bass("TRN2", num_devices=N_CORES)
    Alu = mybir.AluOpType
    F32M = mybir.dt.float32r if mm_r else F32  # score matmul operand dtype
    if mm_r_sums is None:
        mm_r_sums = mm_r
    F32S = mybir.dt.float32r if mm_r_sums else F32  # sums matmul operands

    xa_d = nc.dram_tensor("xa", [N, D + 2], F32, kind="ExternalInput")
    xt_d = nc.dram_tensor("xt", [D, N], F32, kind="ExternalInput")
    c0_d = nc.dram_tensor("c0", [KH, D], F32, kind="ExternalInput")
    c0t_d = nc.dram_tensor("c0t", [D, KH], F32, kind="ExternalInput")
    iota_d = nc.dram_tensor("iota", [P, KH], F32, kind="ExternalInput")
    ident_d = nc.dram_tensor("ident", [P, P], F32, kind="ExternalInput")
    labels_d = nc.dram_tensor("labels", [N], I32, kind="ExternalOutput")
    centers_d = nc.dram_tensor("centers", [KH, D], F32, kind="ExternalOutput")

    with tile.TileContext(nc) as tc:
        with (
            tc.tile_pool(name="res", bufs=1) as res,          # persistent state
            tc.tile_pool(name="work", bufs=4) as work,        # score / onehot
            tc.tile_pool(name="small", bufs=2) as small,      # per-iter misc
            tc.tile_pool(name="ps_xc", bufs=4, space="PSUM") as ps_xc,
            tc.tile_pool(name="ps_sum", bufs=1, space="PSUM") as ps_sum,
            tc.tile_pool(name="ps_aux", bufs=1, space="PSUM") as ps_aux,
            tc.tile_pool(name="dram", bufs=2, space="DRAM") as dram,
        ):
            # ---- resident loads ----
            xa_sb = res.tile([P, NT, D + 2], F32S, tag="xa")
            # point n = t*P + p lives at xa_sb[p, t, :] — matches the score
            # matmul, whose output partition is the lhsT free index.
            xa_v = xa_d.rearrange("(t p) d -> p t d", p=P).bitcast(F32S)
            nc.sync.dma_start(xa_sb[:, :NT // 2], xa_v[:, :NT // 2])
            nc.sync.dma_start(xa_sb[:, NT // 2:], xa_v[:, NT // 2:])
            xt_sb = res.tile([P, DC, N], F32M, tag="xt")
            xt_v = xt_d.rearrange("(c p) n -> p c n", p=P).bitcast(F32M)
            nc.sync.dma_start(xt_sb[:, 0], xt_v[:, 0])
            nc.sync.dma_start(xt_sb[:, 1], xt_v[:, 1])
            cT = res.tile([P, DC, KH], F32M, tag="cT")
            nc.sync.dma_start(
                cT[:], c0t_d.rearrange("(c p) k -> p c k", p=P).bitcast(F32M)
            )
            centers = res.tile([P, 2, D], F32, tag="centers")
            nc.sync.dma_start(
                centers[:], c0_d.rearrange("(kt p) d -> p kt d", p=P)
            )
            iota = res.tile([P, KH], F32, tag="iota")
            nc.sync.dma_start(iota[:], iota_d[:])
            ident = res.tile([P, P], F32, tag="ident")
            nc.sync.dma_start(ident[:], ident_d[:])
            half = res.tile([P, P], F32, tag="half")
            nc.gpsimd.memset(half[:], 0.5)
            c2b = res.tile([P, KH], F32, tag="c2b")
            sq = res.tile([P, DC, KH], F32, tag="sq")
            glabel = res.tile([P, NT], F32, tag="glabel")

            def refresh_c2b():
                # c2b[j, k] = 0.5 * sum_d cT[d, k]^2, broadcast over all
                # partitions j via an all-0.5 stationary matmul.
                cTf = cT[:].bitcast(F32)
                nc.vector.tensor_tensor(sq[:], cTf, cTf, Alu.mult)
                psc = ps_aux.tile([P, KH], F32, tag="psc", name="psc")
                nc.tensor.matmul(
                    psc[:], half[:], sq[:, 0, :], start=True, stop=False
                )
                nc.tensor.matmul(
                    psc[:], half[:], sq[:, 1, :], start=False, stop=True
                )
                nc.vector.tensor_copy(c2b[:], psc[:])

            refresh_c2b()

            for it in range(iters + 1):
                # ---- phase A: scores + local argmax ----
                mx8 = small.tile([P, NT, 8], F32, tag="mx8", name="mx8")
                ix8 = small.tile([P, NT, 8], U32, tag="ix8", name="ix8")
                for t in range(NT):
                    psa = ps_xc.tile([P, KH], F32, tag="psa", name="psa")
                    nc.tensor.matmul(
                        psa[:],
                        xt_sb[:, 0, t * P:(t + 1) * P],
                        cT[:, 0, :],
                        start=True,
                        stop=False,
                    )
                    nc.tensor.matmul(
                        psa[:],
                        xt_sb[:, 1, t * P:(t + 1) * P],
                        cT[:, 1, :],
                        start=False,
                        stop=True,
                    )
                    score = work.tile([P, KH], F32, tag="score", name="score")
                    nc.vector.tensor_tensor(
                        score[:], psa[:], c2b[:], Alu.subtract
                    )
                    nc.vector.max(mx8[:, t, :], score[:])
                    nc.vector.max_index(ix8[:, t, :], mx8[:, t, :], score[:])

                # ---- phase B: pairwise exchange of (maxval, global label) ----
                if dbg_skip_exchange:
                    nc.vector.tensor_copy(glabel[:], mx8[:, :, 0])
                else:
                    xch = small.tile([P, 2, NT], F32, tag="xch", name="xch")
                    nc.vector.tensor_copy(xch[:, 0, :], mx8[:, :, 0])
                    lidx = small.tile([P, NT], F32, tag="lidx", name="lidx")
                    nc.vector.tensor_copy(lidx[:], ix8[:, :, 0])
                    # local -> global cluster index (iota[:, 0] is koff)
                    nc.vector.tensor_scalar(
                        xch[:, 1, :], lidx[:], iota[:, 0:1], None, Alu.add
                    )
                    in_b = dram.tile([P, 2, NT], F32, tag="in_b", name="in_b")
                    out_b = dram.tile([2, P, 2, NT], F32, tag="out_b", name="out_b")
                    nc.sync.dma_start(in_b[:], xch[:])
                    if use_collective:
                        nc.gpsimd.collective_compute(
                            "AllGather",
                            Alu.bypass,
                            replica_groups=REPLICA_GROUPS,
                            ins=[in_b.opt()],
                            outs=[out_b.opt()],
                        )
                    else:  # timing-only variant: fake the exchange locally
                        nc.sync.dma_start(out_b[0], in_b[:])
                        nc.sync.dma_start(out_b[1], in_b[:])
                    pair = small.tile([P, 4, NT], F32, tag="pair", name="pair")
                    nc.sync.dma_start(pair[:, 0:2, :], out_b[0])
                    nc.sync.dma_start(pair[:, 2:4, :], out_b[1])
                    # global argmax: half 1 wins only on strictly greater
                    # score, so exact ties resolve to the lower cluster index.
                    mask = small.tile([P, NT], I32, tag="mask", name="mask")
                    nc.vector.tensor_tensor(
                        mask[:], pair[:, 2, :], pair[:, 0, :], Alu.is_gt
                    )
                    nc.vector.select(
                        glabel[:], mask[:], pair[:, 3, :], pair[:, 1, :]
                    )

                if it == iters:
                    if not dbg_skip_out:
                        li = small.tile([P, NT], I32, tag="li", name="li")
                        nc.vector.tensor_copy(li[:], glabel[:])
                        nc.sync.dma_start(
                            labels_d.rearrange("(t p) -> p t", p=P), li[:]
                        )
                        nc.sync.dma_start(
                            centers_d.rearrange("(kt p) d -> p kt d", p=P),
                            centers[:],
                        )
                    break

                if dbg_skip_cd:
                    continue
                # ---- phase C: one-hot sums (counts fused as ones column) ----
                pss = [
                    ps_sum.tile([P, D + 2], F32, tag=f"pss{kt}", name=f"pss{kt}")
                    for kt in range(2)
                ]
                for t in range(NT):
                    oh = work.tile([P, KH], F32S, tag="oh", name="oh")
                    nc.vector.tensor_scalar(
                        oh[:], iota[:], glabel[:, t:t + 1], None, Alu.is_equal
                    )
                    for kt in range(2):
                        nc.tensor.matmul(
                            pss[kt][:],
                            oh[:, kt * P:(kt + 1) * P],
                            xa_sb[:, t, :],
                            start=(t == 0),
                            stop=(t == NT - 1),
                        )

                # ---- phase D: centroid update ----
                for kt in range(2):
                    counts = pss[kt][:, D:D + 1]
                    cnt1 = small.tile([P, 1], F32, tag="cnt1", name="cnt1")
                    nc.vector.tensor_scalar(
                        cnt1[:], counts, 1.0, None, Alu.max
                    )
                    recip = small.tile([P, 1], F32, tag="recip", name="recip")
                    nc.vector.reciprocal(recip[:], cnt1[:])
                    maskc = small.tile([P, 1], F32, tag="maskc", name="maskc")
                    nc.vector.tensor_scalar(
                        maskc[:], counts, 0.0, None, Alu.is_gt
                    )
                    nmask = small.tile([P, 1], F32, tag="nmask", name="nmask")
                    nc.vector.tensor_scalar(
                        nmask[:], counts, 0.0, None, Alu.is_le
                    )
                    # new = sums/max(counts,1)*(counts>0) + old*(counts<=0)
                    cnew = small.tile([P, D], F32, tag="cnew", name="cnew")
                    nc.vector.tensor_scalar(
                        cnew[:], pss[kt][:, 0:D], recip[:], maskc[:],
                        Alu.mult, Alu.mult,
                    )
                    cold = small.tile([P, D], F32, tag="cold", name="cold")
                    nc.vector.tensor_scalar(
                        cold[:], centers[:, kt, :], nmask[:], None, Alu.mult
                    )
                    nc.vector.tensor_tensor(
                        centers[:, kt, :], cnew[:], cold[:], Alu.add
                    )

                # rebuild cT (transpose) and c2b for the next iteration
                for kt in range(2):
                    for dc in range(DC):
                        pst = ps_aux.tile([P, P], F32, tag="pst", name="pst")
                        nc.tensor.transpose(
                            pst[:],
                            centers[:, kt, dc * P:(dc + 1) * P],
                            ident[:],
                        )
                        nc.vector.tensor_copy(
                            cT[:, dc, kt * P:(kt + 1) * P], pst[:]
                        )
                refresh_c2b()

    _split_multiwait_insts(nc)
    return nc


def kernel(x):
    x = np.ascontiguousarray(np.asarray(x, dtype=np.float32))
    assert x.shape == (B, N, D)
    nc = build()

    in_maps = []
    ident = np.eye(P, dtype=np.float32)
    for c in range(N_CORES):
        b, h = c // 2, c % 2
        xb = x[b]
        koff = h * KH
        in_maps.append(
            {
                "xa": np.ascontiguousarray(
                    np.concatenate(
                        [xb, np.ones((N, 1), np.float32),
                         np.zeros((N, 1), np.float32)], axis=1
                    )
                ),
                "xt": np.ascontiguousarray(xb.T),
                "c0": np.ascontiguousarray(xb[koff:koff + KH]),
                "c0t": np.ascontiguousarray(xb[koff:koff + KH].T),
                "iota": np.tile(
                    np.arange(koff, koff + KH, dtype=np.float32), (P, 1)
                ),
                "ident": ident,
            }
        )

    res = run_bass_kernel_spmd(nc, in_maps, core_ids=list(range(N_CORES)))
    labels = np.stack(
        [res.results[2 * b]["labels"].reshape(N) for b in range(B)]
    ).astype(np.int32)
    centers = np.stack(
        [
            np.concatenate(
                [
                    res.results[2 * b]["centers"],
                    res.results[2 * b + 1]["centers"],
                ],
                axis=0,
            )
            for b in range(B)
        ]
    ).astype(np.float32)
    return labels, centers


# revision 24
# speedup vs baseline: 46.1883x; 1.0525x over previous
"""KMeans (B=4, N=8192, D=256, K=512, 10 Lloyd iterations) on 8 trn2 cores.

Sharding: core c handles batch b = c//2 and cluster half h = c%2 (256
clusters each).  Each core scores all 8192 points of its batch against its
256 clusters, takes a local argmax of (x.c - |c|^2/2), and the two cores of
a batch exchange (maxval, label) via a pairwise AllGather to form the global
argmin labels.  Each core then accumulates per-cluster sums/counts for its
own clusters with a one-hot matmul (counts fused as an extra ones column)
and updates its centroids locally — no all-reduce needed.

Outputs: labels (4, 8192) int32 and centers (4, 512, 256) float32, matching
jnp.argmin tie-breaking (lowest index; cross-half ties resolve to the low
half).
"""

import sys

sys.path.insert(0, "/opt/trn_rl_repo")

import numpy as np

import concourse.bass as bass
import concourse.mybir as mybir
import concourse.tile as tile
from concourse.bass_utils import run_bass_kernel_spmd

F32 = mybir.dt.float32
I32 = mybir.dt.int32
U32 = mybir.dt.uint32

B, N, D, K = 4, 8192, 256, 512
KH = K // 2          # clusters per core
P = 128
NT = N // P          # 64 n-tiles of 128 points
DC = D // P          # 2 contraction chunks
ITERS = 10
N_CORES = 8
REPLICA_GROUPS = [[0, 1], [2, 3], [4, 5], [6, 7]]


def _split_multiwait_insts(nc):
    """This walrus build rejects instructions with >1 semaphore wait
    ("Too many sync wait commands").  Hoist extra waits onto single-wait
    NoOps on the same engine immediately before the instruction —
    semantically identical."""
    n_new = 0
    for f in nc.m.functions:
        for bb in f.blocks:
            out = []
            changed = False
            for inst in bb.instructions:
                si = getattr(inst, "sync_info", None)
                if si is not None and len(si.on_wait) > 1:
                    waits = list(si.on_wait)
                    ups = list(si.on_update)
                    for k, w in enumerate(waits[:-1]):
                        nop = mybir.InstNoOp(
                            name=f"{inst.name}-hw{k}", ins=[], outs=[]
                        )
                        nop.engine = inst.engine
                        nop.sync_info = mybir.SyncInfo(on_wait=[w], on_update=[])
                        out.append(nop)
                        n_new += 1
                    inst.sync_info = mybir.SyncInfo(
                        on_wait=[waits[-1]], on_update=ups
                    )
                    changed = True
                out.append(inst)
            if changed:
                bb.instructions = out
    return n_new


def build(iters=ITERS, use_collective=True, mm_r=False, mm_r_sums=None,
          dbg_skip_exchange=False, dbg_skip_out=False, dbg_skip_cd=False):
    nc = bass.Bass("TRN2", num_devices=N_CORES)
    Alu = mybir.AluOpType
    F32M = mybir.dt.float32r if mm_r else F32  # score matmul operand dtype
    if mm_r_sums is None:
        mm_r_sums = mm_r
    F32S = mybir.dt.float32r if mm_r_sums else F32  # sums matmul operands

    xa_d = nc.dram_tensor("xa", [N, D + 2], F32, kind="ExternalInput")
    xt_d = nc.dram_tensor("xt", [D, N], F32, kind="ExternalInput")
    c0_d = nc.dram_tensor("c0", [KH, D], F32, kind="ExternalInput")
    c0t_d = nc.dram_tensor("c0t", [D, KH], F32, kind="ExternalInput")
    iota_d = nc.dram_tensor("iota", [P, KH], F32, kind="ExternalInput")
    ident_d = nc.dram_tensor("ident", [P, P], F32, kind="ExternalInput")
    labels_d = nc.dram_tensor("labels", [N], I32, kind="ExternalOutput")
    centers_d = nc.dram_tensor("centers", [KH, D], F32, kind="ExternalOutput")

    with tile.TileContext(nc) as tc:
        with (
            tc.tile_pool(name="res", bufs=1) as res,          # persistent state
            tc.tile_pool(name="work", bufs=4) as work,        # score / onehot
            tc.tile_pool(name="small", bufs=2) as small,      # per-iter misc
            tc.tile_pool(name="ps_xc", bufs=4, space="PSUM") as ps_xc,
            tc.tile_pool(name="ps_sum", bufs=1, space="PSUM") as ps_sum,
            tc.tile_pool(name="ps_aux", bufs=1, space="PSUM") as ps_aux,
            tc.tile_pool(name="dram", bufs=2, space="DRAM") as dram,
        ):
            # ---- resident loads ----
            xa_sb = res.tile([P, NT, D + 2], F32S, tag="xa")
            # point n = t*P + p lives at xa_sb[p, t, :] — matches the score
            # matmul, whose output partition is the lhsT free index.
            xa_v = xa_d.rearrange("(t p) d -> p t d", p=P).bitcast(F32S)
            nc.sync.dma_start(xa_sb[:, :NT // 2], xa_v[:, :NT // 2])
            nc.sync.dma_start(xa_sb[:, NT // 2:], xa_v[:, NT // 2:])
            xt_sb = res.tile([P, DC, N], F32M, tag="xt")
            xt_v = xt_d.rearrange("(c p) n -> p c n", p=P).bitcast(F32M)
            nc.sync.dma_start(xt_sb[:, 0], xt_v[:, 0])
            nc.sync.dma_start(xt_sb[:, 1], xt_v[:, 1])
            cT = res.tile([P, DC, KH], F32M, tag="cT")
            nc.sync.dma_start(
                cT[:], c0t_d.rearrange("(c p) k -> p c k", p=P).bitcast(F32M)
            )
            centers = res.tile([P, 2, D], F32, tag="centers")
            nc.sync.dma_start(
                centers[:], c0_d.rearrange("(kt p) d -> p kt d", p=P)
            )
            iota = res.tile([P, KH], F32, tag="iota")
            nc.sync.dma_start(iota[:], iota_d[:])
            ident = res.tile([P, P], F32, tag="ident")
            nc.sync.dma_start(ident[:], ident_d[:])
            half = res.tile([P, P], F32, tag="half")
            nc.gpsimd.memset(half[:], 0.5)
            c2b = res.tile([P, KH], F32, tag="c2b")
            sq = res.tile([P, DC, KH], F32, tag="sq")
            glabel = res.tile([P, NT], F32, tag="glabel")

            def refresh_c2b():
                # c2b[j, k] = 0.5 * sum_d cT[d, k]^2, broadcast over all
                # partitions j via an all-0.5 stationary matmul.
                cTf = cT[:].bitcast(F32)
                nc.vector.tensor_tensor(sq[:], cTf, cTf, Alu.mult)
                psc = ps_aux.tile([P, KH], F32, tag="psc", name="psc")
                nc.tensor.matmul(
                    psc[:], half[:], sq[:, 0, :], start=True, stop=False
                )
                nc.tensor.matmul(
                    psc[:], half[:], sq[:, 1, :], start=False, stop=True
                )
                nc.vector.tensor_copy(c2b[:], psc[:])

            refresh_c2b()

            for it in range(iters + 1):
                # ---- phase A: scores + local argmax ----
                mx8 = small.tile([P, NT, 8], F32, tag="mx8", name="mx8")
                ix8 = small.tile([P, NT, 8], U32, tag="ix8", name="ix8")
                # Two halves: each half's (maxval, label) ships via AllGather
                # as soon as its tiles finish, overlapping the collective with
                # the other half's matmuls (and with phase C of the first
                # half below).
                NH = NT // 2
                out_bs = []
                for h in range(2):
                    for t in range(h * NH, (h + 1) * NH):
                        psa = ps_xc.tile([P, KH], F32, tag="psa", name="psa")
                        nc.tensor.matmul(
                            psa[:],
                            xt_sb[:, 0, t * P:(t + 1) * P],
                            cT[:, 0, :],
                            start=True,
                            stop=False,
                        )
                        nc.tensor.matmul(
                            psa[:],
                            xt_sb[:, 1, t * P:(t + 1) * P],
                            cT[:, 1, :],
                            start=False,
                            stop=True,
                        )
                        score = work.tile([P, KH], F32, tag="score", name="score")
                        nc.vector.tensor_tensor(
                            score[:], psa[:], c2b[:], Alu.subtract
                        )
                        nc.vector.max(mx8[:, t, :], score[:])
                        nc.vector.max_index(ix8[:, t, :], mx8[:, t, :], score[:])
                    if dbg_skip_exchange:
                        continue
                    hs = slice(h * NH, (h + 1) * NH)
                    xch = small.tile([P, 2, NH], F32, tag=f"xch{h}", name=f"xch{h}")
                    nc.vector.tensor_copy(xch[:, 0, :], mx8[:, hs, 0])
                    lidx = small.tile([P, NH], F32, tag=f"lidx{h}", name=f"lidx{h}")
                    nc.vector.tensor_copy(lidx[:], ix8[:, hs, 0])
                    # local -> global cluster index (iota[:, 0] is koff)
                    nc.vector.tensor_scalar(
                        xch[:, 1, :], lidx[:], iota[:, 0:1], None, Alu.add
                    )
                    in_b = dram.tile([P, 2, NH], F32, tag=f"in_b{h}", name=f"in_b{h}")
                    out_b = dram.tile(
                        [2, P, 2, NH], F32, tag=f"out_b{h}", name=f"out_b{h}"
                    )
                    nc.sync.dma_start(in_b[:], xch[:])
                    if use_collective:
                        nc.gpsimd.collective_compute(
                            "AllGather",
                            Alu.bypass,
                            replica_groups=REPLICA_GROUPS,
                            ins=[in_b.opt()],
                            outs=[out_b.opt()],
                        )
                    else:  # timing-only variant: fake the exchange locally
                        nc.sync.dma_start(out_b[0], in_b[:])
                        nc.sync.dma_start(out_b[1], in_b[:])
                    out_bs.append(out_b)

                if dbg_skip_exchange:
                    nc.vector.tensor_copy(glabel[:], mx8[:, :, 0])
                else:
                    for h, out_b in enumerate(out_bs):
                        hs = slice(h * NH, (h + 1) * NH)
                        pair = small.tile(
                            [P, 4, NH], F32, tag=f"pair{h}", name=f"pair{h}"
                        )
                        nc.sync.dma_start(pair[:, 0:2, :], out_b[0])
                        nc.sync.dma_start(pair[:, 2:4, :], out_b[1])
                        # global argmax: half 1 wins only on strictly greater
                        # score, so exact ties resolve to the lower index.
                        mask = small.tile(
                            [P, NH], I32, tag=f"mask{h}", name=f"mask{h}"
                        )
                        nc.vector.tensor_tensor(
                            mask[:], pair[:, 2, :], pair[:, 0, :], Alu.is_gt
                        )
                        nc.vector.select(
                            glabel[:, hs], mask[:], pair[:, 3, :], pair[:, 1, :]
                        )

                if it == iters:
                    if not dbg_skip_out:
                        li = small.tile([P, NT], I32, tag="li", name="li")
                        nc.vector.tensor_copy(li[:], glabel[:])
                        nc.sync.dma_start(
                            labels_d.rearrange("(t p) -> p t", p=P), li[:]
                        )
                        nc.sync.dma_start(
                            centers_d.rearrange("(kt p) d -> p kt d", p=P),
                            centers[:],
                        )
                    break

                if dbg_skip_cd:
                    continue
                # ---- phase C: one-hot sums (counts fused as ones column) ----
                pss = [
                    ps_sum.tile([P, D + 2], F32, tag=f"pss{kt}", name=f"pss{kt}")
                    for kt in range(2)
                ]
                for t in range(NT):
                    oh = work.tile([P, KH], F32S, tag="oh", name="oh")
                    nc.vector.tensor_scalar(
                        oh[:], iota[:], glabel[:, t:t + 1], None, Alu.is_equal
                    )
                    for kt in range(2):
                        nc.tensor.matmul(
                            pss[kt][:],
                            oh[:, kt * P:(kt + 1) * P],
                            xa_sb[:, t, :],
                            start=(t == 0),
                            stop=(t == NT - 1),
                        )

                # ---- phase D: centroid update ----
                for kt in range(2):
                    counts = pss[kt][:, D:D + 1]
                    cnt1 = small.tile([P, 1], F32, tag="cnt1", name="cnt1")
                    nc.vector.tensor_scalar(
                        cnt1[:], counts, 1.0, None, Alu.max
                    )
                    recip = small.tile([P, 1], F32, tag="recip", name="recip")
                    nc.vector.reciprocal(recip[:], cnt1[:])
                    maskc = small.tile([P, 1], F32, tag="maskc", name="maskc")
                    nc.vector.tensor_scalar(
                        maskc[:], counts, 0.0, None, Alu.is_gt
                    )
                    nmask = small.tile([P, 1], F32, tag="nmask", name="nmask")
                    nc.vector.tensor_scalar(
                        nmask[:], counts, 0.0, None, Alu.is_le
                    )
                    # new = sums/max(counts,1)*(counts>0) + old*(counts<=0)
                    cnew = small.tile([P, D], F32, tag="cnew", name="cnew")
                    nc.vector.tensor_scalar(
                        cnew[:], pss[kt][:, 0:D], recip[:], maskc[:],
                        Alu.mult, Alu.mult,
                    )
                    cold = small.tile([P, D], F32, tag="cold", name="cold")
                    nc.vector.tensor_scalar(
                        cold[:], centers[:, kt, :], nmask[:], None, Alu.mult
                    )
                    nc.vector.tensor_tensor(
                        centers[:, kt, :], cnew[:], cold[:], Alu.add
                    )

                # rebuild cT (transpose) and c2b for the next iteration
                for kt in range(2):
                    for dc in range(DC):
                        pst = ps_aux.tile([P, P], F32, tag="pst", name="pst")
                        nc.tensor.transpose(
                            pst[:],
                            centers[:, kt, dc * P:(dc + 1) * P],
                            ident[:],
                        )
                        nc.vector.tensor_copy(
                            cT[:, dc, kt * P:(kt + 1) * P], pst[:]
                        )
                refresh_c2b()

    _split_multiwait_insts(nc)
    return nc


def kernel(x):
    x = np.ascontiguousarray(np.asarray(x, dtype=np.float32))
    assert x.shape == (B, N, D)
    nc = build()

    in_maps = []
    ident = np.eye(P, dtype=np.float32)
    for c in range(N_CORES):
        b, h = c // 2, c % 2
        xb = x[b]
        koff = h * KH
        in_maps.append(
            {
                "xa": np.ascontiguousarray(
                    np.concatenate(
                        [xb, np.ones((N, 1), np.float32),
                         np.zeros((N, 1), np.float32)], axis=1
                    )
                ),
                "xt": np.ascontiguousarray(xb.T),
                "c0": np.ascontiguousarray(xb[koff:koff + KH]),
                "c0t": np.ascontiguousarray(xb[koff:koff + KH].T),
                "iota": np.tile(
                    np.arange(koff, koff + KH, dtype=np.float32), (P, 1)
                ),
                "ident": ident,
            }
        )

    res = run_bass_kernel_spmd(nc, in_maps, core_ids=list(range(N_CORES)))
    labels = np.stack(
        [res.results[2 * b]["labels"].reshape(N) for b in range(B)]
    ).astype(np.int32)
    centers = np.stack(
        [
            np.concatenate(
                [
                    res.results[2 * b]["centers"],
                    res.results[2 * b + 1]["centers"],
                ],
                axis=0,
            )
            for b in range(B)
        ]
    ).astype(np.float32)
    return labels, centers
